# revision 1
# baseline (speedup 1.0000x reference)
"""GQA cross-attention block on 8 trn2 NeuronCores.

Sharding: tensor-parallel over heads. Core c owns KV group g=c (64 dims of
K/V) and its 4 query heads (256 q channels). Each core computes its heads'
attention plus its slice of the o-projection (rows c*256:(c+1)*256 of Wo),
producing a full-shape partial output; the host sums the 8 partials and
adds bo. No device collectives needed.

Device layouts (host prepares):
  xT, encT: [B, HIDDEN, S] bf16 (activations transposed so hidden lands on
  the PE contraction/partition dim), per-core weight slices in bf16,
  biases as [P, 1] fp32 columns for ACT's per-partition bias.

On-chip dataflow per (batch):
  qT [256c, S] = Wq_c^T @ xT   (PE, accum over 16 hidden chunks)
  kT [64, S], vT [64, S] from encT; vT transposed to v_aug [128k, 65]
  chunks with a ones column (row 64 of the AV matmul then yields the
  softmax denominator Z for free).
  scores^T [128k, 512q] = kT-chunk.T @ qT-head-slice (PE)
  E = exp(0.125 * scores) (ACT, PSUM->SBUF bf16)
  av_aug [65, 512q] += v_aug.T @ E (PE, accum over 16 k chunks)
  oT [64, 512q] = av * broadcast(1/Z)  (recip on DVE, broadcast via a
  K=1 PE matmul with a ones row, multiply on DVE)
  out_partial [128s, 512h] = oT.T @ Wo_c (PE) -> bf16 -> DRAM
"""

import numpy as np
import ml_dtypes

import concourse.bass as bass
from concourse import bacc
import concourse.mybir as mybir
import concourse.tile as tile
from concourse.bass_utils import run_bass_kernel_spmd
from concourse.masks import make_identity

BF16 = ml_dtypes.bfloat16
F32 = mybir.dt.float32
BF = mybir.dt.bfloat16

B = 2
S = 2048
HID = 2048
D = 64          # head dim
RQ = 4          # query heads per core (per kv group)
CH = RQ * D     # 256 q channels per core
NCORES = 8
NH = HID // 128  # 16 hidden chunks
NST = S // 512   # 4 s-tiles of 512
NKC = S // 128   # 16 key chunks of 128
SCALE = 1.0 / np.sqrt(D)


def _build_nc() -> bass.Bass:
    nc = bacc.Bacc()

    xT = nc.dram_tensor("xT", [B, HID, S], BF, kind="ExternalInput")
    encT = nc.dram_tensor("encT", [B, HID, S], BF, kind="ExternalInput")
    wq = nc.dram_tensor("wq", [HID, CH], BF, kind="ExternalInput")
    wk = nc.dram_tensor("wk", [HID, D], BF, kind="ExternalInput")
    wv = nc.dram_tensor("wv", [HID, D], BF, kind="ExternalInput")
    wo = nc.dram_tensor("wo", [CH, HID], BF, kind="ExternalInput")
    bq = nc.dram_tensor("bq", [CH, 1], F32, kind="ExternalInput")
    bk = nc.dram_tensor("bk", [D, 1], F32, kind="ExternalInput")
    bv = nc.dram_tensor("bv", [D, 1], F32, kind="ExternalInput")
    out = nc.dram_tensor("out", [B, S, HID], BF, kind="ExternalOutput")

    with tile.TileContext(nc) as tc:
        with (
            tc.tile_pool(name="wpool", bufs=1) as wpool,
            tc.tile_pool(name="xs", bufs=6) as xs_pool,
            tc.tile_pool(name="es", bufs=6) as es_pool,
            tc.tile_pool(name="acts", bufs=2) as acts,
            tc.tile_pool(name="vaug", bufs=2 * NKC) as vaug_pool,
            tc.tile_pool(name="epool", bufs=8) as epool,
            tc.tile_pool(name="small", bufs=4) as small,
            tc.tile_pool(name="osb", bufs=4) as osb_pool,
            tc.tile_pool(name="psum", bufs=2, space="PSUM") as ps,
        ):
            # ---- resident constants / weights ----
            wq_t = []
            wk_t = []
            wv_t = []
            for h in range(NH):
                wqh = wpool.tile([128, CH], BF, name=f"wq{h}")
                nc.sync.dma_start(out=wqh[:], in_=wq[h * 128:(h + 1) * 128, :])
                wq_t.append(wqh)
                wkh = wpool.tile([128, D], BF, name=f"wk{h}")
                nc.sync.dma_start(out=wkh[:], in_=wk[h * 128:(h + 1) * 128, :])
                wk_t.append(wkh)
                wvh = wpool.tile([128, D], BF, name=f"wv{h}")
                nc.sync.dma_start(out=wvh[:], in_=wv[h * 128:(h + 1) * 128, :])
                wv_t.append(wvh)
            wo_t = []
            for cchunk in range(2):
                woc = wpool.tile([128, HID], BF, name=f"wo{cchunk}")
                nc.sync.dma_start(out=woc[:], in_=wo[cchunk * 128:(cchunk + 1) * 128, :])
                wo_t.append(woc)
            bq_t = []
            for cchunk in range(2):
                bqc = wpool.tile([128, 1], F32, name=f"bq{cchunk}")
                nc.sync.dma_start(out=bqc[:], in_=bq[cchunk * 128:(cchunk + 1) * 128, :])
                bq_t.append(bqc)
            bk_t = wpool.tile([D, 1], F32, name="bk_t")
            nc.sync.dma_start(out=bk_t[:], in_=bk[:, :])
            bv_t = wpool.tile([D, 1], F32, name="bv_t")
            nc.sync.dma_start(out=bv_t[:], in_=bv[:, :])

            ident = wpool.tile([128, 128], BF, name="ident")
            make_identity(nc, ident[:])
            ones1 = wpool.tile([1, D], F32, name="ones1")
            nc.gpsimd.memset(ones1[:], 1.0)

            ID = mybir.ActivationFunctionType.Identity
            EXP = mybir.ActivationFunctionType.Exp

            for b in range(B):
                # ---- phase A: projections ----
                qT_h = [
                    acts.tile([D, S], BF, tag=f"q{r}", name=f"q{r}_{b}")
                    for r in range(RQ)
                ]
                kT = acts.tile([D, S], BF, tag="kT", name=f"kT{b}")
                vT = acts.tile([D, S], BF, tag="vT", name=f"vT{b}")

                for st in range(NST):
                    ssl = slice(st * 512, (st + 1) * 512)
                    qps_lo = ps.tile([128, 512], F32, tag="pproj", name=f"qpl{b}{st}")
                    qps_hi = ps.tile([128, 512], F32, tag="pproj", name=f"qph{b}{st}")
                    for h in range(NH):
                        xt = xs_pool.tile([128, 512], BF, tag="xs", name=f"xs{b}{st}{h}")
                        nc.gpsimd.dma_start(
                            out=xt[:], in_=xT[b, h * 128:(h + 1) * 128, ssl])
                        nc.tensor.matmul(
                            qps_lo[:], wq_t[h][:, 0:128], xt[:],
                            start=(h == 0), stop=(h == NH - 1))
                        nc.tensor.matmul(
                            qps_hi[:], wq_t[h][:, 128:256], xt[:],
                            start=(h == 0), stop=(h == NH - 1))
                    for r in range(RQ):
                        src = qps_lo if r < 2 else qps_hi
                        row = (r % 2) * D
                        nc.scalar.activation(
                            qT_h[r][:, ssl], src[row:row + D, :], ID,
                            bias=bq_t[r // 2][row:row + D, :])

                for st in range(NST):
                    ssl = slice(st * 512, (st + 1) * 512)
                    kps = ps.tile([D, 512], F32, tag="pproj", name=f"kps{b}{st}")
                    vps = ps.tile([D, 512], F32, tag="pproj", name=f"vps{b}{st}")
                    for h in range(NH):
                        et = es_pool.tile([128, 512], BF, tag="es", name=f"es{b}{st}{h}")
                        nc.gpsimd.dma_start(
                            out=et[:], in_=encT[b, h * 128:(h + 1) * 128, ssl])
                        nc.tensor.matmul(
                            kps[:], wk_t[h][:], et[:],
                            start=(h == 0), stop=(h == NH - 1))
                        nc.tensor.matmul(
                            vps[:], wv_t[h][:], et[:],
                            start=(h == 0), stop=(h == NH - 1))
                    nc.scalar.activation(kT[:, ssl], kps[:], ID, bias=bk_t[:])
                    nc.scalar.activation(vT[:, ssl], vps[:], ID, bias=bv_t[:])

                # v_aug chunks: [128 kpos, 65] with ones in col 64
                v_aug = []
                for kc in range(NKC):
                    vtp = ps.tile([128, D], BF, tag="ps", name=f"vtp{b}{kc}")
                    nc.tensor.transpose(
                        vtp[:], vT[:, kc * 128:(kc + 1) * 128], ident[0:D, 0:D])
                    va = vaug_pool.tile([128, D + 1], BF, tag="vaug", name=f"va{b}{kc}")
                    nc.gpsimd.memset(va[:, D:D + 1], 1.0)
                    nc.vector.tensor_copy(va[:, 0:D], vtp[:])
                    v_aug.append(va)

                # ---- attention + oT ----
                oT_lo = acts.tile([128, S], BF, tag="olo", name=f"olo{b}")
                oT_hi = acts.tile([128, S], BF, tag="ohi", name=f"ohi{b}")
                for r in range(RQ):
                    odst = oT_lo if r < 2 else oT_hi
                    row = (r % 2) * D
                    avs = [
                        ps.tile([D + 1, 512], F32, tag="pav", bufs=4,
                                name=f"av{b}{r}{qc}")
                        for qc in range(NST)
                    ]
                    # kc-outer, qc pairs inner: stationary (kT chunk /
                    # v_aug chunk) reused across consecutive matmuls, and
                    # all four av accumulators stay hot in PSUM.
                    for kc in range(NKC):
                        ksl = slice(kc * 128, (kc + 1) * 128)
                        for pair in range(NST // 2):
                            scs = []
                            for qc in (2 * pair, 2 * pair + 1):
                                qsl = slice(qc * 512, (qc + 1) * 512)
                                sct = ps.tile([128, 512], F32, tag="ps",
                                              name=f"sc{b}{r}{qc}{kc}")
                                nc.tensor.matmul(
                                    sct[:], kT[:, ksl], qT_h[r][:, qsl],
                                    start=True, stop=True)
                                e_t = epool.tile([128, 512], BF, tag="e",
                                                 name=f"e{b}{r}{qc}{kc}")
                                nc.scalar.activation(
                                    e_t[:], sct[:], EXP, scale=float(SCALE))
                                scs.append(e_t)
                            for j, qc in enumerate((2 * pair, 2 * pair + 1)):
                                nc.tensor.matmul(
                                    avs[qc][:], v_aug[kc][:], scs[j][:],
                                    start=(kc == 0), stop=(kc == NKC - 1))
                    for qc in range(NST):
                        qsl = slice(qc * 512, (qc + 1) * 512)
                        av = avs[qc]
                        rt = small.tile([1, 512], F32, tag="rt", name=f"rt{b}{r}{qc}")
                        nc.vector.reciprocal(rt[:], av[D:D + 1, :])
                        bc = ps.tile([D, 512], F32, tag="ps", name=f"bc{b}{r}{qc}")
                        nc.tensor.matmul(bc[:], ones1[:], rt[:], start=True, stop=True)
                        bcs = small.tile([D, 512], F32, tag="bcs", name=f"bcs{b}{r}{qc}")
                        nc.vector.tensor_copy(bcs[:], bc[:])
                        nc.vector.tensor_mul(odst[row:row + D, qsl], av[0:D, :], bcs[:])

                # ---- o-projection (partial over this core's 256 channels) ----
                for sc16 in range(S // 128):
                    s128 = slice(sc16 * 128, (sc16 + 1) * 128)
                    for hc in range(HID // 512):
                        hsl = slice(hc * 512, (hc + 1) * 512)
                        ops = ps.tile([128, 512], F32, tag="pproj", name=f"op{b}{sc16}{hc}")
                        nc.tensor.matmul(
                            ops[:], oT_lo[:, s128], wo_t[0][:, hsl],
                            start=True, stop=False)
                        nc.tensor.matmul(
                            ops[:], oT_hi[:, s128], wo_t[1][:, hsl],
                            start=False, stop=True)
                        osb = osb_pool.tile([128, 512], BF, tag="osb", name=f"ob{b}{sc16}{hc}")
                        nc.vector.tensor_copy(osb[:], ops[:])
                        nc.sync.dma_start(out=out[b, s128, hsl], in_=osb[:])

    if not nc.is_finalized():
        nc.finalize()
    return nc


_NC = None
_RUN_KWARGS = {}
_LAST_RESULT = None


def _get_nc():
    global _NC
    if _NC is None:
        _NC = _build_nc()
    return _NC


def kernel(x, encoder_output, Wq, bq, Wk, bk, Wv, bv, Wo, bo):
    nc = _get_nc()
    xT = np.ascontiguousarray(np.asarray(x, np.float32).transpose(0, 2, 1)).astype(BF16)
    encT = np.ascontiguousarray(
        np.asarray(encoder_output, np.float32).transpose(0, 2, 1)).astype(BF16)
    Wq = np.asarray(Wq, np.float32)
    Wk = np.asarray(Wk, np.float32)
    Wv = np.asarray(Wv, np.float32)
    Wo = np.asarray(Wo, np.float32)
    in_maps = []
    for c in range(NCORES):
        csl = slice(c * CH, (c + 1) * CH)
        gsl = slice(c * D, (c + 1) * D)
        in_maps.append({
            "xT": xT,
            "encT": encT,
            "wq": np.ascontiguousarray(Wq[:, csl]).astype(BF16),
            "wk": np.ascontiguousarray(Wk[:, gsl]).astype(BF16),
            "wv": np.ascontiguousarray(Wv[:, gsl]).astype(BF16),
            "wo": np.ascontiguousarray(Wo[csl, :]).astype(BF16),
            "bq": np.ascontiguousarray(
                np.asarray(bq, np.float32)[csl].reshape(CH, 1)),
            "bk": np.ascontiguousarray(
                np.asarray(bk, np.float32)[gsl].reshape(D, 1)),
            "bv": np.ascontiguousarray(
                np.asarray(bv, np.float32)[gsl].reshape(D, 1)),
        })
    res = run_bass_kernel_spmd(nc, in_maps, list(range(NCORES)), **_RUN_KWARGS)
    global _LAST_RESULT
    _LAST_RESULT = res
    total = np.zeros((B, S, HID), np.float32)
    for c in range(NCORES):
        total += res.results[c]["out"].astype(np.float32)
    return total + np.asarray(bo, np.float32)



# revision 9
# speedup vs baseline: 1.7601x; 1.7601x over previous
"""GQA cross-attention block on 8 trn2 NeuronCores.

Sharding: tensor-parallel over heads. Core c owns KV group g=c (64 dims of
K/V) and its 4 query heads (256 q channels). Each core computes its heads'
attention plus its slice of the o-projection (rows c*256:(c+1)*256 of Wo),
producing a full-shape partial output; the host sums the 8 partials and
adds bo.

Key structure (v2):
  - K/V projection packed: one stationary [128h, 128] = [Wk_h | Wv_h] per
    hidden chunk -> kvT [128, S] (K rows 0:64, V rows 64:128).
  - Scores row-tiled 2x on the PE: kTd [128, S] holds K duplicated on both
    partition halves; qd2 [128, S] holds a HEAD PAIR (head 2hp on rows
    0:64, head 2hp+1 on rows 64:128). Two concurrent K=64 matmuls
    (tile_position (0,0) and (64,0)) produce scores for both heads into
    one [128, 1024] PSUM pair -> a single [128,1024] exp on ACT.
  - AV via v_aug [128, 65] (ones column -> softmax denominator Z for free
    in row 64), accumulated over 16 key chunks per (head, qtile).
  - Z normalization: av rows evacuated to SBUF fp32; the 8 Z rows per
    head-pair are DMA-gathered into [128, 32], ONE batched reciprocal,
    DMA-scattered back to [1, 2048] rows; 1/Z broadcast via a K=1 PE
    matmul; DVE multiply writes oT.
  - o-projection per 128-row s-chunk with [128, 2048] bf16 staging rows.
  The two batches pipeline: projections of batch 1 fill the ACT-bound PE
  gaps of batch 0's attention, o-proj of batch 0 fills batch 1's.
"""

import numpy as np
import ml_dtypes

import concourse.bass as bass
from concourse import bacc
import concourse.mybir as mybir
import concourse.tile as tile
from concourse.bass_utils import run_bass_kernel_spmd
from concourse.masks import make_identity

BF16 = ml_dtypes.bfloat16
F32 = mybir.dt.float32
BF = mybir.dt.bfloat16

B = 2
S = 2048
HID = 2048
D = 64          # head dim
CH = 4 * D      # 256 q channels per core
NCORES = 8
NH = HID // 128  # 16 hidden chunks
NKC = S // 128   # 16 key chunks of 128
NQC = S // 512   # 4 q tiles of 512
SCALE = 1.0 / np.sqrt(D)


def _build_nc() -> bass.Bass:
    nc = bacc.Bacc()

    xT = nc.dram_tensor("xT", [B, HID, S], BF, kind="ExternalInput")
    encT = nc.dram_tensor("encT", [B, HID, S], BF, kind="ExternalInput")
    wq = nc.dram_tensor("wq", [HID, CH], BF, kind="ExternalInput")
    wkv = nc.dram_tensor("wkv", [HID, 128], BF, kind="ExternalInput")
    wo = nc.dram_tensor("wo", [CH, HID], BF, kind="ExternalInput")
    bq = nc.dram_tensor("bq", [CH, 1], F32, kind="ExternalInput")
    bkv = nc.dram_tensor("bkv", [128, 1], F32, kind="ExternalInput")
    out = nc.dram_tensor("out", [B, S, HID], BF, kind="ExternalOutput")

    ID = mybir.ActivationFunctionType.Identity
    EXP = mybir.ActivationFunctionType.Exp

    with tile.TileContext(nc) as tc:
        with (
            tc.tile_pool(name="wpool", bufs=1) as wpool,
            tc.tile_pool(name="xs", bufs=18) as xs_pool,
            tc.tile_pool(name="es", bufs=5) as es_pool,
            tc.tile_pool(name="acts", bufs=2) as acts,
            tc.tile_pool(name="vaug", bufs=2 * NKC) as vaug_pool,
            tc.tile_pool(name="epool", bufs=3) as epool,
            tc.tile_pool(name="avsb", bufs=8) as avsb_pool,
            tc.tile_pool(name="zp", bufs=2) as zpool,
            tc.tile_pool(name="osb", bufs=2) as osb_pool,
            tc.tile_pool(name="ps_sc", bufs=2, space="PSUM") as ps_sc,
            tc.tile_pool(name="ps_av", bufs=2, space="PSUM") as ps_av,
            tc.tile_pool(name="ps_pr", bufs=2, space="PSUM") as ps_pr,
        ):
            # ---- resident weights / constants ----
            wq_t = []
            wkv_t = []
            for h in range(NH):
                wqh = wpool.tile([128, CH], BF, name=f"wq{h}")
                nc.sync.dma_start(out=wqh[:], in_=wq[h * 128:(h + 1) * 128, :])
                wq_t.append(wqh)
                wkvh = wpool.tile([128, 128], BF, name=f"wkv{h}")
                nc.sync.dma_start(out=wkvh[:], in_=wkv[h * 128:(h + 1) * 128, :])
                wkv_t.append(wkvh)
            wo_t = []
            for cc in range(2):
                woc = wpool.tile([128, HID], BF, name=f"wo{cc}")
                nc.sync.dma_start(out=woc[:], in_=wo[cc * 128:(cc + 1) * 128, :])
                wo_t.append(woc)
            bq_t = []
            for cc in range(2):
                bqc = wpool.tile([128, 1], F32, name=f"bq{cc}")
                nc.sync.dma_start(out=bqc[:], in_=bq[cc * 128:(cc + 1) * 128, :])
                bq_t.append(bqc)
            bkv_t = wpool.tile([128, 1], F32, name="bkv_t")
            nc.sync.dma_start(out=bkv_t[:], in_=bkv[:, :])

            ident = wpool.tile([128, 128], BF, name="ident")
            make_identity(nc, ident[:])
            ones1 = wpool.tile([1, D], BF, name="ones1")
            nc.gpsimd.memset(ones1[:], 1.0)

            for b in range(B):
                # ---- KV projection (h-outer: enc tiles release fast) ----
                kvT = acts.tile([128, S], BF, tag="kvT", bufs=1, name=f"kvT{b}")
                for sh in range(2):
                    kvps = [
                        ps_pr.tile([128, 512], F32, tag="pr", name=f"kvp{b}{sh}{t}")
                        for t in range(2)
                    ]
                    ets = []
                    for h in range(NH):
                        et = es_pool.tile([128, 1024], BF, tag="es",
                                          name=f"es{b}{sh}{h}")
                        nc.gpsimd.dma_start(
                            out=et[:],
                            in_=encT[b, h * 128:(h + 1) * 128,
                                     sh * 1024:(sh + 1) * 1024])
                        ets.append(et)
                    for h in range(NH):
                        for t in range(2):
                            nc.tensor.matmul(
                                kvps[t][:], wkv_t[h][:],
                                ets[h][:, t * 512:(t + 1) * 512],
                                start=(h == 0), stop=(h == NH - 1))
                    for t in range(2):
                        ssl = slice(sh * 1024 + t * 512, sh * 1024 + (t + 1) * 512)
                        nc.scalar.activation(
                            kvT[:, ssl], kvps[t][:], ID, bias=bkv_t[:])

                # kTd: K on both partition halves; vT: V at base partition 0
                kTd = acts.tile([128, S], BF, tag="kTd", name=f"kTd{b}")
                vT = acts.tile([D, S], BF, tag="vT", bufs=1, name=f"vT{b}")
                nc.gpsimd.dma_start(out=kTd[0:D, :], in_=kvT[0:D, :])
                nc.gpsimd.dma_start(out=kTd[D:128, :], in_=kvT[0:D, :])
                nc.gpsimd.dma_start(out=vT[:], in_=kvT[D:128, :])

                # v_aug chunks [128, 65] with ones in col 64
                v_aug = []
                for kc in range(NKC):
                    vtp = ps_pr.tile([128, D], BF, tag="pr", name=f"vtp{b}{kc}")
                    nc.tensor.transpose(
                        vtp[:], vT[:, kc * 128:(kc + 1) * 128], ident[0:D, 0:D])
                    va = vaug_pool.tile([128, D + 1], BF, tag=f"va{kc}",
                                        name=f"va{b}{kc}")
                    nc.gpsimd.memset(va[:, D:D + 1], 1.0)
                    nc.vector.tensor_copy(va[:, 0:D], vtp[:])
                    v_aug.append(va)

                # ---- Q projection -> head-pair tiles qd2[hp] ----
                qd2 = [
                    acts.tile([128, S], BF, tag=f"qd{hp}", name=f"qd{b}{hp}")
                    for hp in range(2)
                ]
                for st in range(4):
                    ssl = slice(st * 512, (st + 1) * 512)
                    xts = []
                    for h in range(NH):
                        xt = xs_pool.tile([128, 512], BF, tag="xs",
                                          name=f"xs{b}{st}{h}")
                        nc.gpsimd.dma_start(
                            out=xt[:],
                            in_=xT[b, h * 128:(h + 1) * 128, ssl])
                        xts.append(xt)
                    qps = [
                        ps_pr.tile([128, 512], F32, tag="pr",
                                   name=f"qp{b}{st}{hp}")
                        for hp in range(2)
                    ]
                    for h in range(NH):
                        for hp in range(2):
                            nc.tensor.matmul(
                                qps[hp][:],
                                wq_t[h][:, hp * 128:(hp + 1) * 128],
                                xts[h][:],
                                start=(h == 0), stop=(h == NH - 1))
                    for hp in range(2):
                        nc.scalar.activation(
                            qd2[hp][:, ssl], qps[hp][:], ID,
                            bias=bq_t[hp][:])

                # ---- attention ----
                oT_t = [
                    acts.tile([128, S], BF, tag=f"oT{hp}", name=f"oT{b}{hp}")
                    for hp in range(2)
                ]
                for hp in range(2):
                    av_sb = []  # (qc, a0, a1)
                    for qc in range(NQC):
                        qsl = slice(qc * 512, (qc + 1) * 512)
                        av0 = ps_av.tile([D + 1, 512], F32, tag="av",
                                         name=f"av0_{b}{hp}{qc}")
                        av1 = ps_av.tile([D + 1, 512], F32, tag="av",
                                         name=f"av1_{b}{hp}{qc}")
                        for kc in range(NKC):
                            ksl = slice(kc * 128, (kc + 1) * 128)
                            sc2 = ps_sc.tile([128, 1024], F32, tag="sc",
                                             name=f"sc{b}{hp}{qc}{kc}")
                            nc.tensor.matmul(
                                sc2[:, 0:512], kTd[0:D, ksl],
                                qd2[hp][0:D, qsl],
                                start=True, stop=True, tile_position=(0, 0))
                            nc.tensor.matmul(
                                sc2[:, 512:1024], kTd[D:128, ksl],
                                qd2[hp][D:128, qsl],
                                start=True, stop=True, tile_position=(64, 0))
                            e2 = epool.tile([128, 1024], BF, tag="e",
                                            name=f"e{b}{hp}{qc}{kc}")
                            nc.scalar.activation(
                                e2[:], sc2[:], EXP, scale=float(SCALE))
                            nc.tensor.matmul(
                                av0[:], v_aug[kc][:], e2[:, 0:512],
                                start=(kc == 0), stop=(kc == NKC - 1))
                            nc.tensor.matmul(
                                av1[:], v_aug[kc][:], e2[:, 512:1024],
                                start=(kc == 0), stop=(kc == NKC - 1))
                        a0 = avsb_pool.tile([D + 1, 512], BF, tag="avsb",
                                            name=f"a0_{b}{hp}{qc}")
                        a1 = avsb_pool.tile([D + 1, 512], BF, tag="avsb",
                                            name=f"a1_{b}{hp}{qc}")
                        nc.vector.tensor_copy(a0[:], av0[:])
                        nc.vector.tensor_copy(a1[:], av1[:])
                        av_sb.append((qc, a0, a1))

                    # Z rows -> [128, 32] -> one reciprocal -> [1, 2048] rows
                    zP = zpool.tile([128, 32], BF, tag="zP", name=f"zP{b}{hp}")
                    for qc, a0, a1 in av_sb:
                        nc.sync.dma_start(
                            out=zP[:, qc * 4:(qc + 1) * 4], in_=a0[D:D + 1, :])
                        nc.sync.dma_start(
                            out=zP[:, 16 + qc * 4:16 + (qc + 1) * 4],
                            in_=a1[D:D + 1, :])
                    rP = zpool.tile([128, 32], BF, tag="rP", name=f"rP{b}{hp}")
                    with nc.allow_low_precision("bf16 1/Z broadcast"):
                        nc.vector.reciprocal(rP[:], zP[:])
                    rrow = [
                        zpool.tile([1, S], BF, tag="rrow", name=f"rr{b}{hp}{j}")
                        for j in range(2)
                    ]
                    for j in range(2):
                        for qc in range(NQC):
                            nc.sync.dma_start(
                                out=rrow[j][:, qc * 512:(qc + 1) * 512],
                                in_=rP[:, j * 16 + qc * 4:j * 16 + (qc + 1) * 4])
                    for qc, a0, a1 in av_sb:
                        qsl = slice(qc * 512, (qc + 1) * 512)
                        for j, av in ((0, a0), (1, a1)):
                            bc = ps_pr.tile([D, 512], F32, tag="pr",
                                            name=f"bc{b}{hp}{qc}{j}")
                            nc.tensor.matmul(
                                bc[:], ones1[:], rrow[j][:, qsl],
                                start=True, stop=True)
                            nc.vector.tensor_mul(
                                oT_t[hp][j * D:(j + 1) * D, qsl],
                                av[0:D, :], bc[:])

                # ---- o-projection (partial over this core's 256 channels) ----
                for sc16 in range(S // 128):
                    s128 = slice(sc16 * 128, (sc16 + 1) * 128)
                    ob = osb_pool.tile([128, HID], BF, tag="osb",
                                       name=f"ob{b}{sc16}")
                    for hc in range(HID // 512):
                        hsl = slice(hc * 512, (hc + 1) * 512)
                        ops = ps_pr.tile([128, 512], F32, tag="pr",
                                         name=f"op{b}{sc16}{hc}")
                        nc.tensor.matmul(
                            ops[:], oT_t[0][:, s128], wo_t[0][:, hsl],
                            start=True, stop=False)
                        nc.tensor.matmul(
                            ops[:], oT_t[1][:, s128], wo_t[1][:, hsl],
                            start=False, stop=True)
                        nc.vector.tensor_copy(ob[:, hsl], ops[:])
                    nc.sync.dma_start(out=out[b, s128, :], in_=ob[:])

    if not nc.is_finalized():
        nc.finalize()
    return nc


_NC = None
_RUN_KWARGS = {}
_LAST_RESULT = None


def _get_nc():
    global _NC
    if _NC is None:
        _NC = _build_nc()
    return _NC


def kernel(x, encoder_output, Wq, bq, Wk, bk, Wv, bv, Wo, bo):
    nc = _get_nc()
    xT = np.ascontiguousarray(
        np.asarray(x, np.float32).transpose(0, 2, 1)).astype(BF16)
    encT = np.ascontiguousarray(
        np.asarray(encoder_output, np.float32).transpose(0, 2, 1)).astype(BF16)
    Wq = np.asarray(Wq, np.float32)
    Wk = np.asarray(Wk, np.float32)
    Wv = np.asarray(Wv, np.float32)
    Wo = np.asarray(Wo, np.float32)
    bq = np.asarray(bq, np.float32)
    bk = np.asarray(bk, np.float32)
    bv = np.asarray(bv, np.float32)
    in_maps = []
    for c in range(NCORES):
        csl = slice(c * CH, (c + 1) * CH)
        gsl = slice(c * D, (c + 1) * D)
        in_maps.append({
            "xT": xT,
            "encT": encT,
            "wq": np.ascontiguousarray(Wq[:, csl]).astype(BF16),
            "wkv": np.ascontiguousarray(
                np.concatenate([Wk[:, gsl], Wv[:, gsl]], axis=1)).astype(BF16),
            "wo": np.ascontiguousarray(Wo[csl, :]).astype(BF16),
            "bq": np.ascontiguousarray(bq[csl].reshape(CH, 1)),
            "bkv": np.ascontiguousarray(
                np.concatenate([bk[gsl], bv[gsl]]).reshape(128, 1)),
        })
    res = run_bass_kernel_spmd(nc, in_maps, list(range(NCORES)), **_RUN_KWARGS)
    global _LAST_RESULT
    _LAST_RESULT = res
    total = np.zeros((B, S, HID), np.float32)
    for c in range(NCORES):
        total += res.results[c]["out"].astype(np.float32)
    return total + np.asarray(bo, np.float32)


# revision 10
# speedup vs baseline: 1.8343x; 1.0422x over previous
"""GQA cross-attention block on 8 trn2 NeuronCores.

Sharding: tensor-parallel over heads. Core c owns KV group g=c (64 dims of
K/V) and its 4 query heads (256 q channels). Each core computes its heads'
attention plus its slice of the o-projection (rows c*256:(c+1)*256 of Wo),
producing a full-shape partial output; the host sums the 8 partials and
adds bo.

Structure (v3):
  - K/V projection packed: stationary [128h, 128] = [Wk_h | Wv_h] ->
    kvT [128, S] (K rows 0:64, V rows 64:128). Evacuated on DVE
    (tensor_scalar_add) so ACT stays reserved for exp.
  - Scores row-tiled 2x on the PE: kTd [128, S] holds K duplicated on both
    partition halves; qd2 [128, S] holds a HEAD PAIR. Two concurrent K=64
    matmuls (tile_position (0,0)/(64,0)) fill one [128, 1024] PSUM pair ->
    a single [128, 1024] exp on ACT (the kernel bottleneck: ~220us/core of
    pure exp streaming).
  - AV via v_aug [128, 65] (ones column -> softmax denominator Z free in
    row 64). Z rows batched into one [128, 32] reciprocal via DMA
    gather/scatter; 1/Z broadcast with a K=1 PE matmul; DVE mul -> oT.
  - Cross-batch software pipelining by interleaved EMISSION: batch 1's
    projection work is emitted in small chunks inside batch 0's attention
    kc-loop (and batch 0's o-projection inside batch 1's attention), so
    the Tile scheduler's priority order alternates and the PE fills the
    ACT-bound gaps. PSUM tags: sc 4 banks, av 2, prj 1, pr 1.
  - DMA spread: enc on gpsimd, x on sync(+scalar at startup), weights on
    scalar, z-dance + stores on sync.
"""

import numpy as np
import ml_dtypes

import concourse.bass as bass
from concourse import bacc
import concourse.mybir as mybir
import concourse.tile as tile
from concourse.bass_utils import run_bass_kernel_spmd
from concourse.masks import make_identity

BF16 = ml_dtypes.bfloat16
F32 = mybir.dt.float32
BF = mybir.dt.bfloat16

B = 2
S = 2048
HID = 2048
D = 64          # head dim
CH = 4 * D      # 256 q channels per core
NCORES = 8
NH = HID // 128  # 16 hidden chunks
NKC = S // 128   # 16 key chunks of 128
NQC = S // 512   # 4 q tiles of 512
NST = S // 512   # 4 s tiles of 512
SCALE = 1.0 / np.sqrt(D)


def _build_nc() -> bass.Bass:
    nc = bacc.Bacc()

    xT = nc.dram_tensor("xT", [B, HID, S], BF, kind="ExternalInput")
    encT = nc.dram_tensor("encT", [B, HID, S], BF, kind="ExternalInput")
    wq = nc.dram_tensor("wq", [HID, CH], BF, kind="ExternalInput")
    wkv = nc.dram_tensor("wkv", [HID, 128], BF, kind="ExternalInput")
    wo = nc.dram_tensor("wo", [CH, HID], BF, kind="ExternalInput")
    bq = nc.dram_tensor("bq", [CH, 1], F32, kind="ExternalInput")
    bkv = nc.dram_tensor("bkv", [128, 1], F32, kind="ExternalInput")
    out = nc.dram_tensor("out", [B, S, HID], BF, kind="ExternalOutput")

    EXP = mybir.ActivationFunctionType.Exp

    with tile.TileContext(nc) as tc:
        with (
            tc.tile_pool(name="wpool", bufs=1) as wpool,
            tc.tile_pool(name="io", bufs=22) as io_pool,
            tc.tile_pool(name="acts", bufs=2) as acts,
            tc.tile_pool(name="vaug", bufs=2 * NKC) as vaug_pool,
            tc.tile_pool(name="epool", bufs=3) as epool,
            tc.tile_pool(name="avsb", bufs=8) as avsb_pool,
            tc.tile_pool(name="zp", bufs=2) as zpool,
            tc.tile_pool(name="osb", bufs=2) as osb_pool,
            tc.tile_pool(name="ps_sc", bufs=2, space="PSUM") as ps_sc,
            tc.tile_pool(name="ps_av", bufs=2, space="PSUM") as ps_av,
            tc.tile_pool(name="ps_prj", bufs=1, space="PSUM") as ps_prj,
            tc.tile_pool(name="ps_pr", bufs=1, space="PSUM") as ps_pr,
        ):
            # ---- resident weights / constants (scalar HWDGE queue) ----
            wkv_t = []
            for h in range(NH):
                wkvh = wpool.tile([128, 128], BF, name=f"wkv{h}")
                nc.scalar.dma_start(out=wkvh[:], in_=wkv[h * 128:(h + 1) * 128, :])
                wkv_t.append(wkvh)
            bkv_t = wpool.tile([128, 1], F32, name="bkv_t")
            nc.scalar.dma_start(out=bkv_t[:], in_=bkv[:, :])
            ident = wpool.tile([128, 128], BF, name="ident")
            make_identity(nc, ident[:])
            wq_t = []
            for h in range(NH):
                wqh = wpool.tile([128, CH], BF, name=f"wq{h}")
                nc.scalar.dma_start(out=wqh[:], in_=wq[h * 128:(h + 1) * 128, :])
                wq_t.append(wqh)
            bq_t = []
            for cc in range(2):
                bqc = wpool.tile([128, 1], F32, name=f"bq{cc}")
                nc.scalar.dma_start(out=bqc[:], in_=bq[cc * 128:(cc + 1) * 128, :])
                bq_t.append(bqc)
            wo_t = []
            for cc in range(2):
                woc = wpool.tile([128, HID], BF, name=f"wo{cc}")
                nc.scalar.dma_start(out=woc[:], in_=wo[cc * 128:(cc + 1) * 128, :])
                wo_t.append(woc)
            ones1 = wpool.tile([1, D], BF, name="ones1")
            nc.gpsimd.memset(ones1[:], 1.0)

            state = {}

            def proj_phase(b, startup):
                """Generator: KV proj, kTd/vT dup, v_aug, Q proj for batch b.
                Yields between small chunks so it can be pumped as PE filler
                inside the other batch's attention loop."""
                st_ = {}
                state[b] = st_
                # --- KV projection ---
                kvT = acts.tile([128, S], BF, tag="kvT", bufs=1, name=f"kvT{b}")
                for st in range(NST):
                    ssl = slice(st * 512, (st + 1) * 512)
                    ets = []
                    for h in range(NH):
                        et = io_pool.tile([128, 512], BF, tag="io",
                                          name=f"es{b}{st}{h}")
                        eng = nc.gpsimd if (startup or h % 2 == 0) else nc.sync
                        eng.dma_start(
                            out=et[:], in_=encT[b, h * 128:(h + 1) * 128, ssl])
                        ets.append(et)
                    yield
                    kvps = ps_prj.tile([128, 512], F32, tag="prj",
                                       name=f"kvp{b}{st}")
                    for h in range(NH):
                        nc.tensor.matmul(
                            kvps[:], wkv_t[h][:], ets[h][:],
                            start=(h == 0), stop=(h == NH - 1))
                        if h % 4 == 3:
                            yield
                    nc.vector.tensor_scalar_add(kvT[:, ssl], kvps[:], bkv_t[:])
                    yield
                # --- kTd (K duplicated on both halves), vT ---
                kTd = acts.tile([128, S], BF, tag="kTd", name=f"kTd{b}")
                vT = acts.tile([D, S], BF, tag="vT", bufs=1, name=f"vT{b}")
                nc.gpsimd.dma_start(out=kTd[0:D, :], in_=kvT[0:D, :])
                nc.gpsimd.dma_start(out=kTd[D:128, :], in_=kvT[0:D, :])
                nc.gpsimd.dma_start(out=vT[:], in_=kvT[D:128, :])
                st_["kTd"] = kTd
                yield
                # --- v_aug chunks [128, 65] with ones in col 64 ---
                v_aug = []
                for kc in range(NKC):
                    vtp = ps_pr.tile([128, D], BF, tag="pr", name=f"vtp{b}{kc}")
                    nc.tensor.transpose(
                        vtp[:], vT[:, kc * 128:(kc + 1) * 128], ident[0:D, 0:D])
                    va = vaug_pool.tile([128, D + 1], BF, tag=f"va{kc}",
                                        name=f"va{b}{kc}")
                    nc.gpsimd.memset(va[:, D:D + 1], 1.0)
                    nc.vector.tensor_copy(va[:, 0:D], vtp[:])
                    v_aug.append(va)
                    if kc % 4 == 3:
                        yield
                st_["va"] = v_aug
                # --- Q projection -> head-pair tiles qd2[hp] ---
                qd2 = [
                    acts.tile([128, S], BF, tag=f"qd{hp}", name=f"qd{b}{hp}")
                    for hp in range(2)
                ]
                st_["qd2"] = qd2
                for st in range(NST):
                    ssl = slice(st * 512, (st + 1) * 512)
                    xts = []
                    for h in range(NH):
                        xt = io_pool.tile([128, 512], BF, tag="io",
                                          name=f"xs{b}{st}{h}")
                        eng = nc.sync if (not startup or h % 2 == 0) else nc.scalar
                        eng.dma_start(
                            out=xt[:], in_=xT[b, h * 128:(h + 1) * 128, ssl])
                        xts.append(xt)
                    yield
                    for hp in range(2):
                        qps = ps_prj.tile([128, 512], F32, tag="prj",
                                          name=f"qp{b}{st}{hp}")
                        for h in range(NH):
                            nc.tensor.matmul(
                                qps[:], wq_t[h][:, hp * 128:(hp + 1) * 128],
                                xts[h][:],
                                start=(h == 0), stop=(h == NH - 1))
                            if h % 4 == 3:
                                yield
                        nc.vector.tensor_scalar_add(
                            qd2[hp][:, ssl], qps[:], bq_t[hp][:])
                        yield

            def pump(gen, n=1):
                if gen is None:
                    return None
                for _ in range(n):
                    try:
                        next(gen)
                    except StopIteration:
                        return None
                return gen

            def attn_phase(b, filler):
                """Attention for batch b; pumps `filler` once per kc step."""
                st_ = state[b]
                kTd, v_aug, qd2 = st_["kTd"], st_["va"], st_["qd2"]
                oT_t = [
                    acts.tile([128, S], BF, tag=f"oT{hp}", name=f"oT{b}{hp}")
                    for hp in range(2)
                ]
                st_["oT"] = oT_t
                for hp in range(2):
                    av_sb = []
                    for qc in range(NQC):
                        qsl = slice(qc * 512, (qc + 1) * 512)
                        av0 = ps_av.tile([D + 1, 512], F32, tag="av",
                                         name=f"av0_{b}{hp}{qc}")
                        av1 = ps_av.tile([D + 1, 512], F32, tag="av",
                                         name=f"av1_{b}{hp}{qc}")
                        for kc in range(NKC):
                            ksl = slice(kc * 128, (kc + 1) * 128)
                            sc2 = ps_sc.tile([128, 1024], F32, tag="sc",
                                             name=f"sc{b}{hp}{qc}{kc}")
                            nc.tensor.matmul(
                                sc2[:, 0:512], kTd[0:D, ksl],
                                qd2[hp][0:D, qsl],
                                start=True, stop=True, tile_position=(0, 0))
                            nc.tensor.matmul(
                                sc2[:, 512:1024], kTd[D:128, ksl],
                                qd2[hp][D:128, qsl],
                                start=True, stop=True, tile_position=(64, 0))
                            e2 = epool.tile([128, 1024], BF, tag="e",
                                            name=f"e{b}{hp}{qc}{kc}")
                            nc.scalar.activation(
                                e2[:], sc2[:], EXP, scale=float(SCALE))
                            nc.tensor.matmul(
                                av0[:], v_aug[kc][:], e2[:, 0:512],
                                start=(kc == 0), stop=(kc == NKC - 1))
                            nc.tensor.matmul(
                                av1[:], v_aug[kc][:], e2[:, 512:1024],
                                start=(kc == 0), stop=(kc == NKC - 1))
                            filler = pump(filler)
                        a0 = avsb_pool.tile([D + 1, 512], BF, tag="avsb",
                                            name=f"a0_{b}{hp}{qc}")
                        a1 = avsb_pool.tile([D + 1, 512], BF, tag="avsb",
                                            name=f"a1_{b}{hp}{qc}")
                        nc.vector.tensor_copy(a0[:], av0[:])
                        nc.vector.tensor_copy(a1[:], av1[:])
                        av_sb.append((qc, a0, a1))

                    # Z rows -> [128, 32] -> one reciprocal -> [1, 2048] rows
                    zP = zpool.tile([128, 32], BF, tag="zP", name=f"zP{b}{hp}")
                    for qc, a0, a1 in av_sb:
                        nc.sync.dma_start(
                            out=zP[:, qc * 4:(qc + 1) * 4], in_=a0[D:D + 1, :])
                        nc.sync.dma_start(
                            out=zP[:, 16 + qc * 4:16 + (qc + 1) * 4],
                            in_=a1[D:D + 1, :])
                    rP = zpool.tile([128, 32], BF, tag="rP", name=f"rP{b}{hp}")
                    with nc.allow_low_precision("bf16 1/Z broadcast"):
                        nc.vector.reciprocal(rP[:], zP[:])
                    rrow = [
                        zpool.tile([1, S], BF, tag="rrow", name=f"rr{b}{hp}{j}")
                        for j in range(2)
                    ]
                    for j in range(2):
                        for qc in range(NQC):
                            nc.sync.dma_start(
                                out=rrow[j][:, qc * 512:(qc + 1) * 512],
                                in_=rP[:, j * 16 + qc * 4:j * 16 + (qc + 1) * 4])
                    for qc, a0, a1 in av_sb:
                        qsl = slice(qc * 512, (qc + 1) * 512)
                        for j, av in ((0, a0), (1, a1)):
                            bc = ps_pr.tile([D, 512], F32, tag="pr",
                                            name=f"bc{b}{hp}{qc}{j}")
                            nc.tensor.matmul(
                                bc[:], ones1[:], rrow[j][:, qsl],
                                start=True, stop=True)
                            nc.vector.tensor_mul(
                                oT_t[hp][j * D:(j + 1) * D, qsl],
                                av[0:D, :], bc[:])
                        filler = pump(filler)
                # drain any remaining filler
                while filler is not None:
                    filler = pump(filler)

            def oproj_phase(b):
                """Generator: o-projection for batch b."""
                oT_t = state[b]["oT"]
                for sc16 in range(S // 128):
                    s128 = slice(sc16 * 128, (sc16 + 1) * 128)
                    ob = osb_pool.tile([128, HID], BF, tag="osb",
                                       name=f"ob{b}{sc16}")
                    for hc in range(HID // 512):
                        hsl = slice(hc * 512, (hc + 1) * 512)
                        ops = ps_pr.tile([128, 512], F32, tag="pr",
                                         name=f"op{b}{sc16}{hc}")
                        nc.tensor.matmul(
                            ops[:], oT_t[0][:, s128], wo_t[0][:, hsl],
                            start=True, stop=False)
                        nc.tensor.matmul(
                            ops[:], oT_t[1][:, s128], wo_t[1][:, hsl],
                            start=False, stop=True)
                        nc.vector.tensor_copy(ob[:, hsl], ops[:])
                        yield
                    nc.sync.dma_start(out=out[b, s128, :], in_=ob[:])
                    yield

            # ---- pipeline ----
            p0 = proj_phase(0, startup=True)
            while pump(p0) is not None:
                pass
            attn_phase(0, filler=proj_phase(1, startup=False))
            o0 = oproj_phase(0)
            attn_phase(1, filler=o0)
            o1 = oproj_phase(1)
            while pump(o1) is not None:
                pass

    if not nc.is_finalized():
        nc.finalize()
    return nc


_NC = None
_RUN_KWARGS = {}
_LAST_RESULT = None


def _get_nc():
    global _NC
    if _NC is None:
        _NC = _build_nc()
    return _NC


def kernel(x, encoder_output, Wq, bq, Wk, bk, Wv, bv, Wo, bo):
    nc = _get_nc()
    xT = np.ascontiguousarray(
        np.asarray(x, np.float32).transpose(0, 2, 1)).astype(BF16)
    encT = np.ascontiguousarray(
        np.asarray(encoder_output, np.float32).transpose(0, 2, 1)).astype(BF16)
    Wq = np.asarray(Wq, np.float32)
    Wk = np.asarray(Wk, np.float32)
    Wv = np.asarray(Wv, np.float32)
    Wo = np.asarray(Wo, np.float32)
    bq = np.asarray(bq, np.float32)
    bk = np.asarray(bk, np.float32)
    bv = np.asarray(bv, np.float32)
    in_maps = []
    for c in range(NCORES):
        csl = slice(c * CH, (c + 1) * CH)
        gsl = slice(c * D, (c + 1) * D)
        in_maps.append({
            "xT": xT,
            "encT": encT,
            "wq": np.ascontiguousarray(Wq[:, csl]).astype(BF16),
            "wkv": np.ascontiguousarray(
                np.concatenate([Wk[:, gsl], Wv[:, gsl]], axis=1)).astype(BF16),
            "wo": np.ascontiguousarray(Wo[csl, :]).astype(BF16),
            "bq": np.ascontiguousarray(bq[csl].reshape(CH, 1)),
            "bkv": np.ascontiguousarray(
                np.concatenate([bk[gsl], bv[gsl]]).reshape(128, 1)),
        })
    res = run_bass_kernel_spmd(nc, in_maps, list(range(NCORES)), **_RUN_KWARGS)
    global _LAST_RESULT
    _LAST_RESULT = res
    total = np.zeros((B, S, HID), np.float32)
    for c in range(NCORES):
        total += res.results[c]["out"].astype(np.float32)
    return total + np.asarray(bo, np.float32)
